# revision 24
# baseline (speedup 1.0000x reference)
"""Trainium2 Bass kernel for the DeformationNetwork MLP.

Data-parallel over 8 NeuronCores: canonical_xyz is sharded along N,
the tiny MLP weights are replicated. Each core runs an identical Tile
program over its 32768-row shard in 512-row tiles.

Device-side layout is feature-on-partition: activations live as
[features, batch] so every layer is matmul(lhsT=W^T chunk, rhs=acts).
The positional sin/cos encoding is built on-device:
  x [128,3] chunks -> broadcast to [128,21] (7 copies of the 3 dims)
  -> PE transpose -> [21, batch] -> multiply by per-partition freqs
  -> one in-place ACT Sin over [36, batch] (cos = sin(x + pi/2)).
The constant time-encoding (21 features, scalar `time`) is folded into
b0/b2 on the host. Head weights (xyz/rot/scale/op) are fused into one
[256,11] matrix; head matmuls use lhsT=h3 so outputs land as
[batch, 11] and DMA out contiguously.
"""

import math

import numpy as np

N_TOTAL = 262144
N_CORES = 8
NROWS = N_TOTAL // N_CORES  # 32768 per core
TILE = 512
NCHUNK = 4  # 128-row chunks per tile
HIDDEN = 256
POS_L = 6
NPOS = 3 + 2 * 3 * POS_L  # 39
NHEAD = 11
NHP = 12  # fp32r matmuls need an even free dim; pad heads 11 -> 12

# matmul input dtype mode: 'f32r' (full-rate, reduced-precision path) or
# 'f32' (exact, 4 cycles/row).
MM_MODE = "f32r"

RND_C = 12582912.0  # 1.5 * 2^23: (u + C) - C == round-to-nearest-even(u)
TWO_PI_LO = float(np.nextafter(np.float32(2 * math.pi), np.float32(0.0)))

_CACHE = {}


def _build(nrows, mm_mode):
    import concourse.mybir as mybir
    import concourse.tile as tile
    from concourse import bacc

    FP = mybir.dt.float32
    AF = mybir.ActivationFunctionType
    ALU = mybir.AluOpType

    ntiles = nrows // TILE

    FPR = mybir.dt.float32r if mm_mode == "f32r" else FP

    nc = bacc.Bacc("TRN2", debug=False)

    # DRAM I/O
    x_d = nc.dram_tensor("x", [nrows, 3], FP, kind="ExternalInput").ap()
    w0t_d = nc.dram_tensor("w0t", [NPOS, HIDDEN], FPR, kind="ExternalInput").ap()
    w1t_d = nc.dram_tensor("w1t", [HIDDEN, HIDDEN], FPR, kind="ExternalInput").ap()
    w2th_d = nc.dram_tensor("w2th", [HIDDEN, HIDDEN], FPR, kind="ExternalInput").ap()
    w2tx_d = nc.dram_tensor("w2tx", [NPOS, HIDDEN], FPR, kind="ExternalInput").ap()
    w3t_d = nc.dram_tensor("w3t", [HIDDEN, HIDDEN], FPR, kind="ExternalInput").ap()
    wht_d = nc.dram_tensor("wht", [HIDDEN, NHP], FPR, kind="ExternalInput").ap()
    b0_d = nc.dram_tensor("b0", [HIDDEN], FP, kind="ExternalInput").ap()
    b1_d = nc.dram_tensor("b1", [HIDDEN], FP, kind="ExternalInput").ap()
    b2_d = nc.dram_tensor("b2", [HIDDEN], FP, kind="ExternalInput").ap()
    b3_d = nc.dram_tensor("b3", [HIDDEN], FP, kind="ExternalInput").ap()
    bh_d = nc.dram_tensor("bh", [128, 4 * NHP], FP, kind="ExternalInput").ap()
    freq_d = nc.dram_tensor("freq39", [NPOS, 1], FP, kind="ExternalInput").ap()
    shift_d = nc.dram_tensor("shift39", [NPOS, 1], FP, kind="ExternalInput").ap()
    id_d = nc.dram_tensor("ident", [128, 128], FP, kind="ExternalInput").ap()
    out_d = nc.dram_tensor("out", [nrows, NHEAD], FP, kind="ExternalOutput").ap()

    with tile.TileContext(nc) as tc:
        from contextlib import ExitStack

        with ExitStack() as ctx:
            wp = ctx.enter_context(tc.tile_pool(name="weights", bufs=1))
            xp = ctx.enter_context(tc.tile_pool(name="x4", bufs=3))
            xap = ctx.enter_context(tc.tile_pool(name="xaug", bufs=2))
            xinp = ctx.enter_context(tc.tile_pool(name="xin", bufs=2))
            hp = ctx.enter_context(tc.tile_pool(name="h", bufs=2))
            op = ctx.enter_context(tc.tile_pool(name="osb", bufs=3))
            pxt = ctx.enter_context(tc.tile_pool(name="pxt", bufs=2, space="PSUM"))
            pbig = ctx.enter_context(tc.tile_pool(name="pbig", bufs=2, space="PSUM"))
            phd = ctx.enter_context(tc.tile_pool(name="phd", bufs=2, space="PSUM"))

            # --- persistent weights ---
            w0t = wp.tile([NPOS, HIDDEN], FPR)
            nc.sync.dma_start(w0t, w0t_d)
            w1t = wp.tile([128, 2, HIDDEN], FPR)
            nc.sync.dma_start(w1t, w1t_d.rearrange("(j p) m -> p j m", p=128))
            w2th = wp.tile([128, 2, HIDDEN], FPR)
            nc.sync.dma_start(w2th, w2th_d.rearrange("(j p) m -> p j m", p=128))
            w2tx = wp.tile([NPOS, HIDDEN], FPR)
            nc.sync.dma_start(w2tx, w2tx_d)
            w3t = wp.tile([128, 2, HIDDEN], FPR)
            nc.sync.dma_start(w3t, w3t_d.rearrange("(j p) m -> p j m", p=128))
            wht = wp.tile([128, 2, NHP], FPR)
            nc.sync.dma_start(wht, wht_d.rearrange("(j p) m -> p j m", p=128))
            b0 = wp.tile([128, 2], FP)
            nc.sync.dma_start(b0, b0_d.rearrange("(j p) -> p j", p=128))
            b1 = wp.tile([128, 2], FP)
            nc.sync.dma_start(b1, b1_d.rearrange("(j p) -> p j", p=128))
            b2 = wp.tile([128, 2], FP)
            nc.sync.dma_start(b2, b2_d.rearrange("(j p) -> p j", p=128))
            b3 = wp.tile([128, 2], FP)
            nc.sync.dma_start(b3, b3_d.rearrange("(j p) -> p j", p=128))
            bh = wp.tile([128, 4 * NHP], FP)
            nc.sync.dma_start(bh, bh_d)
            freqs = wp.tile([NPOS, 1], FP)
            nc.sync.dma_start(freqs, freq_d)
            shifts = wp.tile([NPOS, 1], FP)
            nc.sync.dma_start(shifts, shift_d)
            ident = wp.tile([128, 128], FP)
            nc.sync.dma_start(ident, id_d)

            biases = [b0, b1, b2, b3]

            for t in range(ntiles):
                r0 = t * TILE
                # load 512 rows as 4 chunks of 128 on partitions
                x4 = xp.tile([128, NCHUNK, 3], FP, tag="x4")
                nc.sync.dma_start(
                    x4, x_d[r0 : r0 + TILE, :].rearrange("(c p) d -> p c d", p=128)
                )
                # replicate xyz 13x along free dim: [128, c, 13, 3]
                # rep 0 -> raw xyz; reps 1-6 -> sin rows; reps 7-12 -> cos
                xaug = xap.tile([128, NCHUNK, 13, 3], FP, tag="xaug")
                nc.vector.tensor_copy(
                    xaug, x4[:, :, None, :].to_broadcast((128, NCHUNK, 13, 3))
                )
                # transpose each chunk -> [39, 512] in PSUM
                xt_ps = pxt.tile([NPOS, TILE], FP, tag="xt")
                for c in range(NCHUNK):
                    nc.tensor.transpose(
                        xt_ps[:, c * 128 : (c + 1) * 128], xaug[:, c], ident
                    )
                # xin rows: 0:3 raw xyz, 3:21 sin, 21:39 cos.
                # ACT Sin only covers [-pi, pi], so reduce the argument:
                #   u = x * 2^(l-1) (+0.25 for the cos rows)   [exact]
                #   n = round(u) via +/- 1.5*2^23               [exact]
                #   d = u - n in [-0.5, 0.5]                    [exact]
                #   sin(2*pi*u') = Sin(d * ~2pi)  (scale one ulp under 2pi
                #   so |scaled| <= pi strictly)
                # SBUF engine APs must start at partition 0/32/64/96, so the
                # pipeline runs in partition-0-based tiles and a SBUF->SBUF
                # DMA (no partition alignment rule) drops the result into
                # xin[3:39].
                # Engine APs must start at partition 0/32/64/96, so the whole
                # [39] block runs through the pipeline (freq rows 0:3 are 0,
                # making the raw-x rows harmless zeros) and only rows 3:39
                # are DMA'd into xin.
                xin = xinp.tile([NPOS, TILE], FPR, tag="xin")
                nc.scalar.copy(xin[0:3], xt_ps[0:3])
                u39 = xinp.tile([NPOS, TILE], FP, tag="u39")
                nc.vector.tensor_scalar(u39, xt_ps, freqs, shifts, ALU.mult, ALU.add)
                n39 = xinp.tile([NPOS, TILE], FP, tag="n39")
                nc.vector.tensor_scalar(n39, u39, RND_C, RND_C, ALU.add, ALU.subtract)
                d39 = xinp.tile([NPOS, TILE], FP, tag="d39")
                nc.vector.tensor_tensor(d39, u39, n39, ALU.subtract)
                nc.scalar.activation(d39, d39, AF.Sin, scale=TWO_PI_LO)
                nc.gpsimd.dma_start(xin[3:NPOS], d39[3:NPOS])

                # --- L0: [39] -> [256] ---
                h0_ps = pbig.tile([128, 2, TILE], FP, tag="big")
                for m in range(2):
                    nc.tensor.matmul(
                        h0_ps[:, m],
                        w0t[:, m * 128 : (m + 1) * 128],
                        xin,
                        start=True,
                        stop=True,
                    )
                h0 = hp.tile([128, 2, TILE], FPR, tag="h0")
                for m in range(2):
                    nc.vector.tensor_scalar(
                        h0[:, m], h0_ps[:, m], b0[:, m : m + 1], 0.0, ALU.add, ALU.max
                    )

                # --- L1: [256] -> [256] ---
                h1_ps = pbig.tile([128, 2, TILE], FP, tag="big")
                for m in range(2):
                    for j in range(2):
                        nc.tensor.matmul(
                            h1_ps[:, m],
                            w1t[:, j, m * 128 : (m + 1) * 128],
                            h0[:, j],
                            start=(j == 0),
                            stop=(j == 1),
                        )
                h1 = hp.tile([128, 2, TILE], FPR, tag="h1")
                for m in range(2):
                    nc.scalar.activation(
                        h1[:, m], h1_ps[:, m], AF.Relu, bias=b1[:, m : m + 1]
                    )

                # --- L2 (skip layer): [256 h + 39 x] -> [256] ---
                h2_ps = pbig.tile([128, 2, TILE], FP, tag="big")
                for m in range(2):
                    msl = slice(m * 128, (m + 1) * 128)
                    nc.tensor.matmul(
                        h2_ps[:, m], w2th[:, 0, msl], h1[:, 0],
                        start=True, stop=False,
                    )
                    nc.tensor.matmul(
                        h2_ps[:, m], w2th[:, 1, msl], h1[:, 1],
                        start=False, stop=False,
                    )
                    nc.tensor.matmul(
                        h2_ps[:, m], w2tx[:, msl], xin,
                        start=False, stop=True,
                    )
                h2 = hp.tile([128, 2, TILE], FPR, tag="h2")
                nc.vector.tensor_scalar(
                    h2[:, 0], h2_ps[:, 0], b2[:, 0:1], 0.0, ALU.add, ALU.max
                )
                nc.scalar.activation(h2[:, 1], h2_ps[:, 1], AF.Relu, bias=b2[:, 1:2])

                # --- L3: [256] -> [256] ---
                h3_ps = pbig.tile([128, 2, TILE], FP, tag="big")
                for m in range(2):
                    for j in range(2):
                        nc.tensor.matmul(
                            h3_ps[:, m],
                            w3t[:, j, m * 128 : (m + 1) * 128],
                            h2[:, j],
                            start=(j == 0),
                            stop=(j == 1),
                        )
                h3 = hp.tile([128, 2, TILE], FPR, tag="h3")
                for m in range(2):
                    nc.scalar.activation(
                        h3[:, m], h3_ps[:, m], AF.Relu, bias=b3[:, m : m + 1]
                    )

                # --- heads: out[batch, 11] = h3_chunk.T @ WhT ---
                hd_ps = phd.tile([128, NCHUNK * NHP], FP, tag="hd")
                for q in range(NCHUNK):
                    qs = slice(q * NHP, (q + 1) * NHP)
                    for j in range(2):
                        nc.tensor.matmul(
                            hd_ps[:, qs],
                            h3[:, j, q * 128 : (q + 1) * 128],
                            wht[:, j],
                            start=(j == 0),
                            stop=(j == 1),
                        )
                out_sb = op.tile([128, NCHUNK * NHP], FP, tag="osb")
                nc.vector.tensor_tensor(out_sb, hd_ps, bh, ALU.add)
                for q in range(NCHUNK):
                    nc.sync.dma_start(
                        out_d[r0 + q * 128 : r0 + (q + 1) * 128, :],
                        out_sb[:, q * NHP : q * NHP + NHEAD],
                    )
    nc.compile()
    return nc


def _get_nc(nrows, mm_mode):
    key = (nrows, mm_mode)
    if key not in _CACHE:
        _CACHE[key] = _build(nrows, mm_mode)
    return _CACHE[key]


def _host_prep(inputs):
    """Permute/fold weights on the host. Returns the shared in_map dict."""
    f = {k: np.asarray(v, np.float64) for k, v in inputs.items() if k != "canonical_xyz"}

    # reference feature order inside pos encoding (39):
    #   [x0,x1,x2, d0:(s0..s5,c0..c5), d1:(...), d2:(...)]
    # our row order: [x0,x1,x2, sin l-major (l,d), cos l-major (l,d)]
    ref_cols = np.zeros(NPOS, np.int64)
    ref_cols[0:3] = [0, 1, 2]
    for l in range(POS_L):
        for d in range(3):
            ref_cols[3 + 3 * l + d] = 3 + 12 * d + l
            ref_cols[21 + 3 * l + d] = 3 + 12 * d + 6 + l

    # constant time encoding (21) folded into b0 / b2
    t = float(np.asarray(inputs["time"]).reshape(-1)[0])
    tf = (2.0 ** np.arange(10, dtype=np.float64)) * math.pi
    te = np.concatenate([[t], np.sin(t * tf), np.cos(t * tf)])

    W0, W1, W2, W3 = f["W0"], f["W1"], f["W2"], f["W3"]
    b0 = f["b0"] + W0[:, 39:60] @ te
    b2 = f["b2"] + W2[:, 256 + 39 : 256 + 60] @ te

    whall = np.concatenate([f["Wxyz"], f["Wrot"], f["Wscale"], f["Wop"]], axis=0)
    bhall = np.concatenate([f["bxyz"], f["brot"], f["bscale"], f["bop"]])

    # u = x * 2^(l-1): arg/(2*pi) for arg = x * 2^l * pi. Cos rows get +0.25
    # (cos(a) = sin(a + pi/2)). Rows 0:3 (raw xyz) get freq 0 -> zeros.
    freq39 = np.zeros(NPOS, np.float64)
    shift39 = np.zeros(NPOS, np.float64)
    for l in range(POS_L):
        freq39[3 + 3 * l : 6 + 3 * l] = 2.0 ** (l - 1)
        freq39[21 + 3 * l : 24 + 3 * l] = 2.0 ** (l - 1)
        shift39[21 + 3 * l : 24 + 3 * l] = 0.25

    F = np.float32
    return {
        "w0t": np.ascontiguousarray(W0[:, ref_cols].T, F),
        "w1t": np.ascontiguousarray(W1.T, F),
        "w2th": np.ascontiguousarray(W2[:, :256].T, F),
        "w2tx": np.ascontiguousarray(W2[:, 256 + ref_cols].T, F),
        "w3t": np.ascontiguousarray(W3.T, F),
        "wht": np.ascontiguousarray(np.pad(whall, ((0, 1), (0, 0))).T, F),
        "b0": b0.astype(F),
        "b1": f["b1"].astype(F),
        "b2": b2.astype(F),
        "b3": f["b3"].astype(F),
        "bh": np.tile(np.pad(bhall, (0, 1)).astype(F), (128, 4)),
        "freq39": freq39.astype(F).reshape(NPOS, 1),
        "shift39": shift39.astype(F).reshape(NPOS, 1),
        "ident": np.eye(128, dtype=F),
    }


def run(inputs, trace=False):
    """Run on 8 cores. Returns ((dxyz, drot, dscale, dop), exec_time_ns)."""
    from concourse import bass_utils

    x = np.ascontiguousarray(np.asarray(inputs["canonical_xyz"], np.float32))
    assert x.shape == (N_TOTAL, 3)
    shared = _host_prep(inputs)
    nc = _get_nc(NROWS, MM_MODE)
    in_maps = [
        {**shared, "x": np.ascontiguousarray(x[i * NROWS : (i + 1) * NROWS])}
        for i in range(N_CORES)
    ]
    res = bass_utils.run_bass_kernel_spmd(
        nc, in_maps, core_ids=list(range(N_CORES)), trace=trace
    )
    full = np.concatenate([res.results[i]["out"] for i in range(N_CORES)], axis=0)
    outs = (full[:, 0:3], full[:, 3:7], full[:, 7:10], full[:, 10:11])
    return outs, res.exec_time_ns


def kernel(**inputs):
    outs, _ = run(inputs, trace=False)
    return outs
